# revision 1
# baseline (speedup 1.0000x reference)
"""Trainium2 Bass kernel for a 3-branch GCN layer (sum of three GCNConvs).

Math: out[b,t,:,:] = sum_k A_k @ (x[b,t] @ W_k) + b_k, where A_k is the
symmetric-normalized adjacency (with self loops) of the k-th tiny graph.
Since N=25 nodes and C=64 channels are small and the graphs are shared
across the whole (B,T) batch, the whole operator collapses into one
[1600 x 1600] matrix applied to x rows: out_row = x_row @ Mop + bias,
with Mop = sum_k kron(A_k^T, W_k) precomputed on host.

Device side (data-parallel over batch across 8 NeuronCores): x is cast
to fp16 on the host, each core streams its [2400, 1600] row block,
transposes 128-row tiles on the PE (identity matmul), and accumulates
psum[bt, out-slice] over the 13 K-chunks with fp16 matmuls (fp32 psum
accumulate) against SBUF-resident fp16 Mop chunks. This is a
[2400 x 1600 x 1600] GEMM per core running at ~95% of the PE
column-streaming rate; the bias is added on the DVE during the
psum->SBUF copy-out.
"""

import sys

import numpy as np

if "/opt/trn_rl_repo" not in sys.path:
    sys.path.insert(0, "/opt/trn_rl_repo")

B, T, NNODES, C = 64, 300, 25, 64
F = NNODES * C  # 1600
N_CORES = 8
BT_LOC = (B // N_CORES) * T  # 2400

_PROGRAM_CACHE = {}
# extra kwargs for run_bass_kernel_spmd (test harness sets trace=True here)
_RUN_KW = {}


def _dense_adj(edge_index_k: np.ndarray) -> np.ndarray:
    """PyG GCNConv normalized dense adjacency A[dst, src] (float64)."""
    row = edge_index_k[0].astype(np.int64)
    col = edge_index_k[1].astype(np.int64)
    loop = np.arange(NNODES, dtype=np.int64)
    row = np.concatenate([row, loop])
    col = np.concatenate([col, loop])
    deg = np.zeros(NNODES, dtype=np.float64)
    np.add.at(deg, col, 1.0)
    dinv = np.where(deg > 0, 1.0 / np.sqrt(deg), 0.0)
    norm = dinv[row] * dinv[col]
    A = np.zeros((NNODES, NNODES), dtype=np.float64)
    np.add.at(A, (col, row), norm)
    return A


def _chunks(total, step):
    return [(s, min(step, total - s)) for s in range(0, total, step)]


def _build_program():
    import concourse.bass as bass
    import concourse.tile as tile
    from concourse import bacc, mybir

    f32 = mybir.dt.float32
    f32r = mybir.dt.float32r
    f16 = mybir.dt.float16

    nc = bacc.Bacc(
        "TRN2", target_bir_lowering=False, debug=False, num_devices=N_CORES
    )
    x = nc.dram_tensor("x", [BT_LOC, F], f16, kind="ExternalInput").ap()
    out = nc.dram_tensor("out", [BT_LOC, F], f32, kind="ExternalOutput").ap()
    mop = nc.dram_tensor("mop", [F, F], f16, kind="ExternalInput").ap()
    biasrow = nc.dram_tensor("biasrow", [128, F], f32, kind="ExternalInput").ap()
    ident = nc.dram_tensor("ident", [128, 128], f16, kind="ExternalInput").ap()

    KCH = _chunks(F, 128)       # 13 chunks: 12x128 + 64
    ROWS = _chunks(BT_LOC, 128)  # 19 tiles: 18x128 + 96
    NSL = _chunks(F, 400)       # 4 slices of 400 (>=256 keeps f32r at 1 cyc/row)

    with tile.TileContext(nc) as tc:
        with (
            tc.tile_pool(name="const", bufs=1) as const_pool,
            tc.tile_pool(name="xin", bufs=6) as xin_pool,
            tc.tile_pool(name="xT", bufs=6) as xT_pool,
            tc.tile_pool(name="outp", bufs=3) as out_pool,
            tc.tile_pool(name="tp", bufs=4, space="PSUM") as tp_pool,
            tc.tile_pool(name="po", bufs=1, space="PSUM") as po_pool,
        ):
# preload constants on the scalar HWDGE queue so they run at full
            # DMA rate without queuing ahead of the x-tile streaming DMAs
            ident_sb = const_pool.tile([128, 128], f16, tag="ident")
            nc.sync.dma_start(ident_sb[:], ident[:])
            mop_sb = []
            for kc, (k0, kn) in enumerate(KCH):
                t = const_pool.tile([kn, F], f16, tag=f"mop{kc}")
                nc.scalar.dma_start(t[:], mop[k0 : k0 + kn, :])
                mop_sb.append(t)
            bias_sb = const_pool.tile([128, F], f32, tag="bias")
            nc.scalar.dma_start(bias_sb[:], biasrow[:])

            def emit_transposes(t, r0, rn):
                # x is pre-cast to fp16 on the host, so tiles land ready for
                # the 1 cyc/row PE transposes with no on-chip cast pass
                xt16 = xin_pool.tile([128, F], f16, tag="x")
                nc.sync.dma_start(xt16[:rn], x[r0 : r0 + rn, :])
                xTs = []
                for kc, (k0, kn) in enumerate(KCH):
                    tp = tp_pool.tile([128, 128], f16, tag="tp")
                    nc.tensor.transpose(
                        tp[:kn, :rn], xt16[:rn, k0 : k0 + kn], ident_sb[:rn, :rn]
                    )
                    xT = xT_pool.tile([128, 128], f16, tag=f"xT{kc}")
                    if kc % 2 == 0:
                        nc.scalar.copy(xT[:kn, :rn], tp[:kn, :rn])
                    else:
                        nc.vector.tensor_copy(xT[:kn, :rn], tp[:kn, :rn])
                    xTs.append(xT)
                return xTs

            def emit_matmuls(r0, rn, xTs):
                outt = out_pool.tile([128, F], f32, tag="o")
                nkc = len(KCH)
                pos = [
                    po_pool.tile([128, 400], f32, tag=f"po{s}", name=f"po{s}")
                    for s in range(len(NSL))
                ]
                # k-outer: one weight load per xT chunk, reused across N-slices
                for i, (k0, kn) in enumerate(KCH):
                    for s, (s0, sn) in enumerate(NSL):
                        nc.tensor.matmul(
                            pos[s][:rn, :sn],
                            xTs[i][:kn, :rn],
                            mop_sb[i][:, s0 : s0 + sn],
                            start=(i == 0),
                            stop=(i == nkc - 1),
                        )
                for s, (s0, sn) in enumerate(NSL):
                    nc.vector.tensor_add(
                        outt[:rn, s0 : s0 + sn],
                        pos[s][:rn, :sn],
                        bias_sb[:rn, s0 : s0 + sn],
                    )
                    nc.sync.dma_start(
                        out[r0 : r0 + rn, s0 : s0 + sn], outt[:rn, s0 : s0 + sn]
                    )

            # software pipeline: transposes run ahead of matmuls so
            # (a) PE has transpose work to do while the Mop preload streams
            # in at kernel start, (b) weight loads never wait on a
            # just-finished psum->sbuf copy.
            DEPTH = 5
            pending = []
            for t, (r0, rn) in enumerate(ROWS):
                xTs = emit_transposes(t, r0, rn)
                pending.append((r0, rn, xTs))
                if len(pending) >= DEPTH:
                    emit_matmuls(*pending.pop(0))
            while pending:
                emit_matmuls(*pending.pop(0))

    nc.compile()
    return nc


def kernel(x, edge_index, W1, W2, W3, b1, b2, b3):
    from concourse.bass_utils import run_bass_kernel_spmd

    x = np.ascontiguousarray(np.asarray(x, dtype=np.float32).astype(np.float16))
    edge_index = np.asarray(edge_index)
    Ws = [np.asarray(W, dtype=np.float64) for W in (W1, W2, W3)]
    bs = [np.asarray(b, dtype=np.float64) for b in (b1, b2, b3)]

    Mop = np.zeros((F, F), dtype=np.float64)
    bias = np.zeros(C, dtype=np.float64)
    for k in range(3):
        A = _dense_adj(edge_index[k])
        Mop += np.kron(A.T, Ws[k])
        bias += bs[k]
    Mop16 = Mop.astype(np.float16)
    biasrow = np.broadcast_to(
        np.tile(bias, NNODES).astype(np.float32)[None, :], (128, F)
    ).copy()
    ident = np.eye(128, dtype=np.float16)

    if "nc" not in _PROGRAM_CACHE:
        _PROGRAM_CACHE["nc"] = _build_program()
    nc = _PROGRAM_CACHE["nc"]

    xs = x.reshape(N_CORES, BT_LOC, F)
    in_maps = [
        {
            "x": xs[i],
            "mop": Mop16,
            "biasrow": biasrow,
            "ident": ident,
        }
        for i in range(N_CORES)
    ]
    res = run_bass_kernel_spmd(nc, in_maps, list(range(N_CORES)), **_RUN_KW)
    _PROGRAM_CACHE["last_result"] = res
    out = np.concatenate(
        [res.results[i]["out"][None] for i in range(N_CORES)], axis=0
    )
    return np.ascontiguousarray(
        out.reshape(B, T, NNODES, C).astype(np.float32)
    )

